# revision 20
# baseline (speedup 1.0000x reference)
"""Decoder layer (attn + FFN + 2 layernorms) on 8 Trainium2 cores.

Sharding: core c handles batch b = c//4, query chunk i = c%4 (512 tokens).
Each core redundantly computes K/V for the full sequence (communication-free).
Causality is handled by rotating the key/value token order per core on the
host (self chunk first, then past, then future) so the mask structure is
uniform across cores: k-tiles 0-3 (the self chunk) get compile-time
triangular masks, the rest get a per-core additive bias (0 for past,
-1e30 for future) folded into the softmax exp. Softmax runs unnormalized
(no max subtraction; scores are O(+-8)) with the denominator taken from an
appended ones-column on V, and the division folded into the context copy.

K/V are computed and consumed chunk-by-chunk (fused with attention) so they
never need full SBUF residency; per-head context accumulates in SBUF across
chunks. The FFN intermediate (d_ff=4096) bounces through DRAM.

All matmuls run in float32r (TF32-like fast fp32 mode: 1 cycle/row at
free-dim >= 256 vs 4 cycles/row for exact fp32).
"""

import sys

sys.path.insert(0, "/opt/trn_rl_repo")

import numpy as np

D = 1024          # d_model
H = 16            # heads
HD = 64           # head dim
DFF = 4096
EPS = 1e-6
B, S = 2, 2048
QCH = 512         # query tokens per core
NCORES = 8
P = 128
NCH = S // QCH            # 4 chunks of k/v tokens
KT_TILES = S // P         # 16 k tiles
NDT = D // P              # 8 d_model tiles
NFT = DFF // P            # 32 d_ff tiles
QT_T = QCH // P           # 4 query token tiles
NEG = -1.0e30

_CACHE = {}


def _build(mm_dtype_name="float32r", debug=False):
    import concourse.bacc as bacc
    import concourse.mybir as mybir
    import concourse.tile as tile
    from concourse.masks import make_identity

    dt = mybir.dt
    MMDT = getattr(dt, mm_dtype_name)
    AF = mybir.ActivationFunctionType
    OP = mybir.AluOpType

    nc = bacc.Bacc("TRN2", target_bir_lowering=False, debug=False)

    # ---- I/O ----
    xb = nc.dram_tensor("xb", [S, D], dt.float32, kind="ExternalInput")
    xq = nc.dram_tensor("xq", [QCH, D], dt.float32, kind="ExternalInput")
    kbias = nc.dram_tensor("kbias", [P, KT_TILES], dt.float32, kind="ExternalInput")
    Wq = nc.dram_tensor("Wq", [D, D], dt.float32, kind="ExternalInput")
    Wk = nc.dram_tensor("Wk", [D, D], dt.float32, kind="ExternalInput")
    Wv = nc.dram_tensor("Wv", [D, D], dt.float32, kind="ExternalInput")
    Wo = nc.dram_tensor("Wo", [D, D], dt.float32, kind="ExternalInput")
    W1 = nc.dram_tensor("W1", [D, DFF], dt.float32, kind="ExternalInput")
    W2 = nc.dram_tensor("W2", [DFF, D], dt.float32, kind="ExternalInput")
    bq = nc.dram_tensor("bq", [D], dt.float32, kind="ExternalInput")
    bk = nc.dram_tensor("bk", [D], dt.float32, kind="ExternalInput")
    bv = nc.dram_tensor("bv", [D], dt.float32, kind="ExternalInput")
    bo = nc.dram_tensor("bo", [D], dt.float32, kind="ExternalInput")
    b1 = nc.dram_tensor("b1", [DFF], dt.float32, kind="ExternalInput")
    b2 = nc.dram_tensor("b2", [D], dt.float32, kind="ExternalInput")
    g1 = nc.dram_tensor("g1", [D], dt.float32, kind="ExternalInput")
    be1 = nc.dram_tensor("be1", [D], dt.float32, kind="ExternalInput")
    g2 = nc.dram_tensor("g2", [D], dt.float32, kind="ExternalInput")
    be2 = nc.dram_tensor("be2", [D], dt.float32, kind="ExternalInput")
    out = nc.dram_tensor("out", [QCH, D], dt.float32, kind="ExternalOutput")
    dbg = {}
    if debug:
        for nm, shp in [("dbg_xqT", [P, NDT, QCH]), ("dbg_QT", [P, NDT, QCH]),
                        ("dbg_kt", [P, NDT, QCH]), ("dbg_v", [P, QT_T, H, HD + 1]),
                        ("dbg_ctx", [P, NDT, QCH]), ("dbg_cs", [P, 4, QCH]),
                        ("dbg_ctxT", [P, NDT, QCH]), ("dbg_yT", [P, NDT, QCH]),
                        ("dbg_hT", [P, NDT, QCH]), ("dbg_ex", [P, QCH])]:
            dbg[nm] = nc.dram_tensor(nm, shp, dt.float32, kind="ExternalOutput")

    xb3 = xb.rearrange("(c t p) d -> c t p d", t=QT_T, p=P)  # chunk, toktile, p, d
    xq3 = xq.rearrange("(t p) d -> t p d", p=P)
    WqT = Wq.rearrange("(ko p) d -> p ko d", p=P)  # d_in on partitions
    WkT = Wk.rearrange("(ko p) d -> p ko d", p=P)
    WvT = Wv.rearrange("(ko p) d -> p ko d", p=P)
    WoT = Wo.rearrange("(ko p) d -> p ko d", p=P)
    W1T = W1.rearrange("(ko p) f -> p ko f", p=P)
    W2T = W2.rearrange("(ko p) d -> p ko d", p=P)

    with tile.TileContext(nc) as tc:
        with (
            tc.tile_pool(name="consts", bufs=1) as consts,
            tc.tile_pool(name="mid", bufs=4) as mid,
            tc.tile_pool(name="ktb", bufs=1) as ktb_pool,
            tc.tile_pool(name="vb", bufs=1) as vb_pool,
            tc.tile_pool(name="wraw", bufs=2) as wraw,
            tc.tile_pool(name="wrnd", bufs=2) as wrnd,
            tc.tile_pool(name="expp", bufs=2) as expp,
            tc.tile_pool(name="small", bufs=2) as small,
            tc.tile_pool(name="small1", bufs=1) as small1,
            tc.tile_pool(name="ffs", bufs=2) as ffs,
            tc.tile_pool(name="dram", bufs=1, space="DRAM") as dram,
            tc.tile_pool(name="ps_mm", bufs=4, space="PSUM") as ps_mm,
            tc.tile_pool(name="ps_tp", bufs=1, space="PSUM") as ps_tp,
            tc.tile_pool(name="ps_sc", bufs=2, space="PSUM") as ps_sc,
            tc.tile_pool(name="ps_ctx", bufs=1, space="PSUM") as ps_ctx,
        ):
            # ---- constants ----
            ident = consts.tile([P, P], dt.float32, tag="ident")
            make_identity(nc, ident[:])
            scr32 = consts.tile([P, QCH], dt.float32, tag="scr32")
            ones_r = consts.tile([P, P], MMDT, tag="ones")
            nc.vector.memset(scr32[:], 1.0)
            nc.vector.tensor_copy(out=ones_r[:], in_=scr32[:, 0:P])
            ones64 = consts.tile([P, HD], MMDT, tag="ones64")
            nc.vector.tensor_copy(out=ones64[:], in_=scr32[:, 0:HD])
            tri = consts.tile([P, QT_T, QCH], MMDT, tag="tri")
            for j in range(QT_T):
                # keep where f - p - 128j >= 0  <=>  (128j + p) <= f
                nc.vector.memset(scr32[:], 1.0)
                nc.gpsimd.affine_select(
                    out=scr32[:], in_=scr32[:],
                    compare_op=OP.is_ge, fill=0.0,
                    base=-P * j, pattern=[[1, QCH]], channel_multiplier=-1,
                )
                nc.vector.tensor_copy(out=tri[:, j, :], in_=scr32[:])
            kbias_sb = consts.tile([P, KT_TILES], dt.float32, tag="kbias")
            nc.sync.dma_start(kbias_sb[:], kbias[:])
            eps_sb = consts.tile([P, 1], dt.float32, tag="eps")
            nc.vector.memset(eps_sb[:], EPS)

            def load_vec_pd(name, ap, n):
                t = consts.tile([P, n], dt.float32, tag=name)
                nc.sync.dma_start(t[:], ap.rearrange("(o p) -> p o", p=P))
                return t

            bq_sb = load_vec_pd("bq", bq, NDT)
            bk_sb = load_vec_pd("bk", bk, NDT)
            bo_sb = load_vec_pd("bo", bo, NDT)
            b1_sb = load_vec_pd("b1", b1, NFT)
            b2_sb = load_vec_pd("b2", b2, NDT)
            g1_sb = load_vec_pd("g1", g1, NDT)
            be1_sb = load_vec_pd("be1", be1, NDT)
            g2_sb = load_vec_pd("g2", g2, NDT)
            be2_sb = load_vec_pd("be2", be2, NDT)
            bv_sb = consts.tile([P, D], dt.float32, tag="bv")
            nc.gpsimd.dma_start(out=bv_sb[:], in_=bv[None, :].to_broadcast([P, D]))
            colsum = consts.tile([P, 4, QCH], MMDT, tag="colsum")
            nc.vector.memset(scr32[:], 0.0)
            for _s in range(4):
                nc.vector.tensor_copy(out=colsum[:, _s, :], in_=scr32[:])

            ff_dram = dram.tile([NFT, P, QCH], MMDT)

            def transpose_in(src_ap, dst_tile, dst_do, dst_cols):
                """dst[:, dst_do, dst_cols] = (128x128 fp32 block).T via PE."""
                pt = ps_tp.tile([P, P], dt.float32, tag="ps_tp")
                nc.tensor.transpose(pt[:], src_ap, ident[:])
                nc.vector.tensor_copy(out=dst_tile[:, dst_do, dst_cols], in_=pt[:])

            def stream_round(dram_ap, shape, tag):
                raw = wraw.tile(shape, dt.float32, tag=tag)
                nc.sync.dma_start(raw[:], dram_ap)
                rnd = wrnd.tile(shape, MMDT, tag=tag + "_r")
                nc.vector.tensor_copy(out=rnd[:], in_=raw[:])
                return rnd

            # ---- phase Q: transpose xq, project Q ----
            xqT = mid.tile([P, NDT, QCH], MMDT, tag="mid", name="xqT")
            for t in range(QT_T):
                xn = small.tile([P, D], dt.float32, tag="xnat")
                nc.sync.dma_start(xn[:], xq3[t])
                for do in range(NDT):
                    transpose_in(xn[:, do * P:(do + 1) * P], xqT, do,
                                 slice(t * P, (t + 1) * P))
            QT = mid.tile([P, NDT, QCH], MMDT, tag="mid", name="QT")
            for do in range(NDT):
                wq_r = stream_round(WqT[:, :, do * P:(do + 1) * P],
                                    [P, NDT, P], "wsm")
                pq = ps_mm.tile([P, QCH], dt.float32, tag="ps_mm")
                for k in range(NDT):
                    nc.tensor.matmul(pq[:], wq_r[:, k, :], xqT[:, k, :],
                                     start=(k == 0), stop=(k == NDT - 1))
                nc.vector.tensor_scalar(
                    out=QT[:, do, :], in0=pq[:], scalar1=bq_sb[:, do:do + 1],
                    scalar2=None, op0=OP.add)

            if debug:
                nc.sync.dma_start(dbg["dbg_xqT"][:], xqT[:].bitcast(dt.float32))
                nc.sync.dma_start(dbg["dbg_QT"][:], QT[:].bitcast(dt.float32))

            # ---- fused K/V projection + attention, chunk by chunk ----
            ctx_sb = mid.tile([P, NDT, QCH], dt.float32, tag="mid", name="ctx_sb")
            nc.vector.memset(ctx_sb[:], 0.0)
            for c in range(NCH):
                xTc = mid.tile([P, NDT, QCH], MMDT, tag="mid", name="xTc")
                for t in range(QT_T):
                    xn = small.tile([P, D], dt.float32, tag="xnat")
                    nc.sync.dma_start(xn[:], xb3[c, t])
                    for do in range(NDT):
                        transpose_in(xn[:, do * P:(do + 1) * P], xTc, do,
                                     slice(t * P, (t + 1) * P))
                # K block: [d_out, 512 k-tokens]
                ktblk = ktb_pool.tile([P, NDT, QCH], MMDT, tag="ktb")
                for do in range(NDT):
                    wk_r = stream_round(WkT[:, :, do * P:(do + 1) * P],
                                        [P, NDT, P], "wsm")
                    pk = ps_mm.tile([P, QCH], dt.float32, tag="ps_mm")
                    for k in range(NDT):
                        nc.tensor.matmul(pk[:], wk_r[:, k, :], xTc[:, k, :],
                                         start=(k == 0), stop=(k == NDT - 1))
                    nc.vector.tensor_scalar(
                        out=ktblk[:, do, :], in0=pk[:],
                        scalar1=bk_sb[:, do:do + 1], scalar2=None, op0=OP.add)
                # V block: [tok, head, 64+1] with ones column
                vblk = vb_pool.tile([P, QT_T, H, HD + 1], MMDT, tag="vb")
                nc.vector.tensor_copy(out=vblk[:, :, :, HD], in_=ones64[:])
                for nh in range(2):
                    pvs = [ps_mm.tile([P, QCH], dt.float32, tag="ps_mm",
                                      name=f"pv{t}") for t in range(QT_T)]
                    for k in range(NDT):
                        wv_r = stream_round(WvT[:, k, nh * QCH:(nh + 1) * QCH],
                                            [P, QCH], "wv")
                        for t in range(QT_T):
                            nc.tensor.matmul(
                                pvs[t][:], xTc[:, k, t * P:(t + 1) * P], wv_r[:],
                                start=(k == 0), stop=(k == NDT - 1))
                    for t in range(QT_T):
                        nc.vector.tensor_tensor(
                            vblk[:, t, nh * 8:(nh + 1) * 8, 0:HD],
                            pvs[t][:].rearrange("p (h d) -> p h d", d=HD),
                            bv_sb[:, nh * QCH:(nh + 1) * QCH].rearrange(
                                "p (h d) -> p h d", d=HD),
                            OP.add)
                if debug and c == 0:
                    nc.sync.dma_start(dbg["dbg_kt"][:], ktblk[:].bitcast(dt.float32))
                    nc.sync.dma_start(dbg["dbg_v"][:], vblk[:].bitcast(dt.float32))
                # attention for this k block
                for h in range(H):
                    dti, bp = h // 2, (h % 2) * HD
                    pc = ps_ctx.tile([P, QCH], dt.float32, tag="ps_ctx")
                    for j in range(QT_T):
                        ktg = c * QT_T + j
                        psc = ps_sc.tile([P, QCH], dt.float32, tag="ps_sc")
                        nc.tensor.matmul(
                            psc[:], ktblk[bp:bp + HD, dti, j * P:(j + 1) * P],
                            QT[bp:bp + HD, dti, :], start=True, stop=True)
                        ex = expp.tile([P, QCH], MMDT, tag="exp")
                        nc.scalar.activation(
                            out=ex[:], in_=psc[:], func=AF.Exp,
                            bias=kbias_sb[:, ktg:ktg + 1], scale=0.125)
                        if c == 0:
                            nc.vector.tensor_tensor(ex[:], ex[:], tri[:, j, :],
                                                    OP.mult)
                        if debug and c == 0 and h == 0 and j == 0:
                            nc.sync.dma_start(dbg["dbg_ex"][:],
                                              ex[:].bitcast(dt.float32))
                        nc.tensor.matmul(
                            pc[0:HD + 1, :], vblk[:, j, h, :], ex[:],
                            start=(j == 0), stop=(j == QT_T - 1))
                    nc.vector.tensor_tensor(
                        ctx_sb[bp:bp + HD, dti, :], ctx_sb[bp:bp + HD, dti, :],
                        pc[0:HD, :], OP.add)
                    cb, cs = 32 * (h % 4), h // 4
                    nc.vector.tensor_tensor(
                        colsum[cb:cb + 1, cs, :], colsum[cb:cb + 1, cs, :],
                        pc[HD:HD + 1, :], OP.add)

            if debug:
                nc.sync.dma_start(dbg["dbg_ctx"][:], ctx_sb[:])
                nc.sync.dma_start(dbg["dbg_cs"][:], colsum[:].bitcast(dt.float32))
            # normalize context -> f32r: reciprocal colsum, then broadcast
            # each head's row across partitions via a K=1 ones matmul in PSUM
            with nc.allow_low_precision(reason="f32r recip colsum, ~1e-4 ok"):
                nc.vector.reciprocal(out=colsum[:], in_=colsum[:])
            ctxT = mid.tile([P, NDT, QCH], MMDT, tag="mid", name="ctxT")
            for h in range(H):
                dti, bp = h // 2, (h % 2) * HD
                cb, cs = 32 * (h % 4), h // 4
                prc = ps_sc.tile([P, QCH], dt.float32, tag="ps_sc")
                nc.tensor.matmul(prc[:], ones_r[cb:cb + 1, :],
                                 colsum[cb:cb + 1, cs, :], start=True, stop=True,
                                 tile_position=(cb, 0))
                nc.vector.tensor_tensor(
                    ctxT[bp:bp + HD, dti, :], ctx_sb[bp:bp + HD, dti, :],
                    prc[bp:bp + HD, :], OP.mult)

            # ---- O-proj + residual + LN1 ----
            yT = mid.tile([P, NDT, QCH], MMDT, tag="mid", name="yT")
            for do in range(NDT):
                wo_r = stream_round(WoT[:, :, do * P:(do + 1) * P],
                                    [P, NDT, P], "wsm")
                po = ps_mm.tile([P, QCH], dt.float32, tag="ps_mm")
                for k in range(NDT):
                    nc.tensor.matmul(po[:], wo_r[:, k, :], ctxT[:, k, :],
                                     start=(k == 0), stop=(k == NDT - 1))
                nc.vector.scalar_tensor_tensor(
                    out=yT[:, do, :], in0=po[:], scalar=bo_sb[:, do:do + 1],
                    in1=xqT[:, do, :], op0=OP.add, op1=OP.add)

            def layer_norm(src, dst, g_sb, be_sb):
                """dst[:, do, :] = LN(src) over d_model (partition + do axes);
                per-token (free-axis) stats via ones-matmul column sums."""
                ps1 = ps_mm.tile([P, QCH], dt.float32, tag="ps_mm")
                for do in range(NDT):
                    nc.tensor.matmul(ps1[:], ones_r[:], src[:, do, :],
                                     start=(do == 0), stop=(do == NDT - 1))
                ps2 = ps_mm.tile([P, QCH], dt.float32, tag="ps_mm")
                for do in range(NDT):
                    sq = small1.tile([P, QCH], MMDT, tag="sq")
                    nc.vector.tensor_tensor(sq[:], src[:, do, :], src[:, do, :],
                                            OP.mult)
                    nc.tensor.matmul(ps2[:], ones_r[:], sq[:],
                                     start=(do == 0), stop=(do == NDT - 1))
                mean = small1.tile([P, QCH], MMDT, tag="mean")
                nc.vector.tensor_scalar(out=mean[:], in0=ps1[:], scalar1=1.0 / D,
                                        scalar2=None, op0=OP.mult)
                m2 = small1.tile([P, QCH], MMDT, tag="m2")
                nc.vector.tensor_tensor(m2[:], mean[:], mean[:], OP.mult)
                var = small1.tile([P, QCH], MMDT, tag="var")
                nc.vector.scalar_tensor_tensor(
                    out=var[:], in0=ps2[:], scalar=1.0 / D, in1=m2[:],
                    op0=OP.mult, op1=OP.subtract)
                sstd = small1.tile([P, QCH], MMDT, tag="sstd")
                nc.scalar.activation(out=sstd[:], in_=var[:], func=AF.Sqrt,
                                     bias=eps_sb[:], scale=1.0)
                rstd = small1.tile([P, QCH], MMDT, tag="rstd")
                with nc.allow_low_precision(reason="f32r rstd, ~1e-4 rel ok"):
                    nc.vector.reciprocal(out=rstd[:], in_=sstd[:])
                for do in range(NDT):
                    t1 = small.tile([P, QCH], MMDT, tag="ln_t1")
                    nc.vector.tensor_tensor(t1[:], src[:, do, :], mean[:],
                                            OP.subtract)
                    nc.vector.tensor_tensor(t1[:], t1[:], rstd[:], OP.mult)
                    nc.vector.tensor_scalar(
                        out=dst[:, do, :], in0=t1[:],
                        scalar1=g_sb[:, do:do + 1], scalar2=be_sb[:, do:do + 1],
                        op0=OP.mult, op1=OP.add)

            hT = mid.tile([P, NDT, QCH], MMDT, tag="mid", name="hT")
            layer_norm(yT, hT, g1_sb, be1_sb)
            if debug:
                nc.sync.dma_start(dbg["dbg_ctxT"][:], ctxT[:].bitcast(dt.float32))
                nc.sync.dma_start(dbg["dbg_yT"][:], yT[:].bitcast(dt.float32))
                nc.sync.dma_start(dbg["dbg_hT"][:], hT[:].bitcast(dt.float32))

            # ---- FFN (d_ff intermediate bounces through DRAM) ----
            for ft in range(NFT):
                w1_r = stream_round(W1T[:, :, ft * P:(ft + 1) * P],
                                    [P, NDT, P], "wsm")
                pf = ps_mm.tile([P, QCH], dt.float32, tag="ps_mm")
                for k in range(NDT):
                    nc.tensor.matmul(pf[:], w1_r[:, k, :], hT[:, k, :],
                                     start=(k == 0), stop=(k == NDT - 1))
                ffo = ffs.tile([P, QCH], MMDT, tag="ffo")
                nc.scalar.activation(out=ffo[:], in_=pf[:], func=AF.Relu,
                                     bias=b1_sb[:, ft:ft + 1], scale=1.0)
                nc.sync.dma_start(ff_dram[ft], ffo[:])
            y2T = mid.tile([P, NDT, QCH], MMDT, tag="mid", name="y2T")
            for dog in range(2):
                pds = [ps_mm.tile([P, QCH], dt.float32, tag="ps_mm",
                                  name=f"pd{d4}") for d4 in range(4)]
                for k in range(NFT):
                    ffi = ffs.tile([P, QCH], MMDT, tag="ffi")
                    nc.sync.dma_start(ffi[:], ff_dram[k])
                    w2_r = stream_round(W2T[:, k, dog * QCH:(dog + 1) * QCH],
                                        [P, QCH], "wv")
                    for d4 in range(4):
                        nc.tensor.matmul(
                            pds[d4][:], w2_r[:, d4 * P:(d4 + 1) * P], ffi[:],
                            start=(k == 0), stop=(k == NFT - 1))
                for d4 in range(4):
                    do = dog * 4 + d4
                    nc.vector.scalar_tensor_tensor(
                        out=y2T[:, do, :], in0=pds[d4][:],
                        scalar=b2_sb[:, do:do + 1], in1=hT[:, do, :],
                        op0=OP.add, op1=OP.add)

            outT = mid.tile([P, NDT, QCH], dt.float32, tag="mid", name="outT")
            layer_norm(y2T, outT, g2_sb, be2_sb)

            # ---- transpose back, DMA out ----
            out3 = out.rearrange("(t p) d -> t p d", p=P)
            for t in range(QT_T):
                on = small.tile([P, D], dt.float32, tag="xnat")
                for do in range(NDT):
                    pt = ps_tp.tile([P, P], dt.float32, tag="ps_tp")
                    nc.tensor.transpose(pt[:], outT[:, do, t * P:(t + 1) * P],
                                        ident[:])
                    nc.vector.tensor_copy(out=on[:, do * P:(do + 1) * P], in_=pt[:])
                nc.sync.dma_start(out3[t], on[:])

    nc.finalize()
    return nc


def _get_nc(mm_dtype_name="float32r", debug=False):
    key = ("nc", mm_dtype_name, debug)
    if key not in _CACHE:
        _CACHE[key] = _build(mm_dtype_name, debug)
    return _CACHE[key]


def kernel(x, mask, Wq, bq, Wk, bk, Wv, bv, Wo, bo, W1, b1, W2, b2,
           gamma1, beta1, gamma2, beta2, _trace=False, _mm_dtype="float32r",
           _debug=False):
    from concourse.bass_utils import run_bass_kernel_spmd

    nc = _get_nc(_mm_dtype, _debug)
    x = np.ascontiguousarray(np.asarray(x, dtype=np.float32))
    shared = {
        "Wq": np.asarray(Wq, np.float32), "Wk": np.asarray(Wk, np.float32),
        "Wv": np.asarray(Wv, np.float32), "Wo": np.asarray(Wo, np.float32),
        "W1": np.asarray(W1, np.float32), "W2": np.asarray(W2, np.float32),
        "bq": np.asarray(bq, np.float32), "bk": np.asarray(bk, np.float32),
        "bv": np.asarray(bv, np.float32), "bo": np.asarray(bo, np.float32),
        "b1": np.asarray(b1, np.float32), "b2": np.asarray(b2, np.float32),
        "g1": np.asarray(gamma1, np.float32), "be1": np.asarray(beta1, np.float32),
        "g2": np.asarray(gamma2, np.float32), "be2": np.asarray(beta2, np.float32),
    }
    in_maps = []
    for c in range(NCORES):
        b, i = divmod(c, NCORES // B)
        q0 = i * QCH
        xb_rot = np.concatenate(
            [x[b, q0:q0 + QCH], x[b, :q0], x[b, q0 + QCH:]], axis=0)
        kb = np.zeros((P, KT_TILES), np.float32)
        n_ok = QT_T + q0 // P  # self tiles + past tiles
        kb[:, n_ok:] = NEG
        in_maps.append({
            "xb": np.ascontiguousarray(xb_rot),
            "xq": np.ascontiguousarray(x[b, q0:q0 + QCH]),
            "kbias": kb,
            **shared,
        })
    res = run_bass_kernel_spmd(nc, in_maps, core_ids=list(range(NCORES)),
                               trace=_trace)
    out = np.empty((B, S, D), np.float32)
    for c in range(NCORES):
        b, i = divmod(c, NCORES // B)
        out[b, i * QCH:(i + 1) * QCH] = res.results[c]["out"]
    if _trace:
        _CACHE["last_result"] = res
    return out
